# revision 7
# baseline (speedup 1.0000x reference)
"""BoxConv2d Trainium2 kernel.

Math: the reference computes, per output channel k = (c, f),
    out[b,k] = interp-row(I) diff  then  interp-col diff
where I is the zero-padded integral image of input[b,c].  The whole
pipeline (integral image + fractional box-edge interpolation) is linear
in the input and separable, so it collapses to

    out[b,k] = A_k @ x[b,c] @ B_k^T

with banded "pixel overlap" matrices
    A_k[xo, a] = clamp(xo + x_max_k + 1 - a, 0, 1) - clamp(xo + x_min_k - a, 0, 1)
    B_k[yo, j] = clamp(yo + y_max_k + 1 - j, 0, 1) - clamp(yo + y_min_k - j, 0, 1)

(A_k[xo, a] is exactly the length of the overlap between the box row
extent [xo + x_min, xo + x_max + 1] and the pixel row [a, a+1]; same for
columns.)  A/B are built on the host from the tiny (C,F) box params and
shipped per-core; the device does pure 128x128 matmuls on the PE array.

Sharding: output channels K = C*F = 128 are split across 8 cores
(16 channels = 4 in_planes per core).  Each core reads only its 4 input
planes, so input reads are not duplicated across the chip.
"""

import sys

if "/opt/trn_rl_repo" not in sys.path:
    sys.path.insert(0, "/opt/trn_rl_repo")

import numpy as np

import concourse.bass as bass  # noqa: F401
import concourse.mybir as mybir
import concourse.tile as tile
from concourse import bacc
from concourse.bass_utils import run_bass_kernel_spmd

B, C, F, H, W = 8, 32, 4, 128, 128
NCORES = 8
CPC = C // NCORES  # in_planes per core
KPC = CPC * F      # output channels per core

_DT = mybir.dt.float32
# matmul-operand dtype: float32r streams fp32 through the PE at full rate
# (vs 4 cycles/row for plain float32)
import os as _os
_MM_DT = {
    "f32": mybir.dt.float32,
    "f32r": mybir.dt.float32r,
}[_os.environ.get("BOXCONV_MM_DT", "f32r")]

_NC_CACHE = {}
LAST_RESULT = None


def _build_nc():
    nc = bacc.Bacc(
        "TRN2", target_bir_lowering=False, debug=False, num_devices=NCORES
    )
    x_p = nc.declare_dram_parameter("x", [B, H, CPC * W], _MM_DT, isOutput=False)
    at_p = nc.declare_dram_parameter("at", [H, KPC * H], _MM_DT, isOutput=False)
    bt_p = nc.declare_dram_parameter("bt", [W, KPC * W], _MM_DT, isOutput=False)
    out_p = nc.declare_dram_parameter("out", [B, H, KPC * W], _DT, isOutput=True)

    with tile.TileContext(nc) as tc:
        with (
            tc.tile_pool(name="const", bufs=1) as cpool,
            tc.tile_pool(name="xin", bufs=2) as xpool,
            tc.tile_pool(name="vsb", bufs=3) as vpool,
            tc.tile_pool(name="osb", bufs=3) as opool,
            tc.tile_pool(name="pv", bufs=2, space="PSUM") as pvpool,
            tc.tile_pool(name="po", bufs=2, space="PSUM") as popool,
        ):
            at_sb = cpool.tile([128, KPC * H], _MM_DT, tag="at")
            nc.sync.dma_start(at_sb[:], at_p[:])
            bt_sb = cpool.tile([128, KPC * W], _MM_DT, tag="bt")
            nc.sync.dma_start(bt_sb[:], bt_p[:])

            for b in range(B):
                x_sb = xpool.tile([128, CPC * W], _MM_DT)
                nc.sync.dma_start(x_sb[:], x_p[b])
                for c in range(CPC):
                    # pass 1: V[j, (f,xo)] = sum_a x[a, j] * A_k[xo, a]
                    v_ps = pvpool.tile([128, F * H], mybir.dt.float32)
                    nc.tensor.matmul(
                        v_ps[:],
                        lhsT=x_sb[:, c * W:(c + 1) * W],
                        rhs=at_sb[:, c * F * H:(c + 1) * F * H],
                        start=True,
                        stop=True,
                    )
                    v_sb = vpool.tile([128, F * H], _MM_DT)
                    nc.vector.tensor_copy(v_sb[:], v_ps[:])
                    # pass 2: out[xo, yo] = sum_j V[j, xo] * B_k[yo, j]
                    o_ps = popool.tile([128, F * W], mybir.dt.float32)
                    for f in range(F):
                        kl = c * F + f
                        nc.tensor.matmul(
                            o_ps[:, f * W:(f + 1) * W],
                            lhsT=v_sb[:, f * H:(f + 1) * H],
                            rhs=bt_sb[:, kl * W:(kl + 1) * W],
                            start=True,
                            stop=True,
                        )
                    o_sb = opool.tile([128, F * W], _DT)
                    nc.vector.tensor_copy(o_sb[:], o_ps[:])
                    nc.sync.dma_start(
                        out_p[b][:, c * F * W:(c + 1) * F * W], o_sb[:]
                    )
    nc.finalize()
    return nc


def _get_nc():
    if "nc" not in _NC_CACHE:
        _NC_CACHE["nc"] = _build_nc()
    return _NC_CACHE["nc"]


def _overlap_mats(lo, hi):
    """(K, out, in) pixel-overlap matrices for 128-wide axis."""
    t = np.arange(128, dtype=np.float64)
    d = t[:, None] - t[None, :]  # out - in
    lo = lo.astype(np.float64)[:, None, None]
    hi = hi.astype(np.float64)[:, None, None]
    m = np.clip(d[None] + hi + 1.0, 0.0, 1.0) - np.clip(d[None] + lo, 0.0, 1.0)
    return m.astype(np.float32)


def _make_in_maps(input, x_min, x_max, y_min, y_max):
    A = _overlap_mats(x_min.reshape(-1), x_max.reshape(-1))   # (K, xo, a)
    Bm = _overlap_mats(y_min.reshape(-1), y_max.reshape(-1))  # (K, yo, j)
    in_maps = []
    for m in range(NCORES):
        ks = slice(KPC * m, KPC * (m + 1))
        at = A[ks].transpose(2, 0, 1).reshape(H, KPC * H)     # [a, (kl, xo)]
        bt = Bm[ks].transpose(2, 0, 1).reshape(W, KPC * W)    # [j, (kl, yo)]
        xm = input[:, CPC * m:CPC * (m + 1)].transpose(0, 2, 1, 3)
        xm = xm.reshape(B, H, CPC * W)                        # [b, a, (c, j)]
        in_maps.append({
            "x": np.ascontiguousarray(xm, dtype=np.float32),
            "at": np.ascontiguousarray(at, dtype=np.float32),
            "bt": np.ascontiguousarray(bt, dtype=np.float32),
        })
    return in_maps


def _assemble(results):
    out = np.empty((B, C * F, H, W), np.float32)
    for m in range(NCORES):
        o = results[m]["out"].reshape(B, H, KPC, W).transpose(0, 2, 1, 3)
        out[:, KPC * m:KPC * (m + 1)] = o
    return out


def _run(inputs, trace=False):
    global LAST_RESULT
    nc = _get_nc()
    in_maps = _make_in_maps(**inputs)
    LAST_RESULT = run_bass_kernel_spmd(
        nc, in_maps, list(range(NCORES)), trace=trace
    )
    return _assemble(LAST_RESULT.results)


def kernel(input, x_min, x_max, y_min, y_max):
    return _run({
        "input": np.asarray(input),
        "x_min": np.asarray(x_min),
        "x_max": np.asarray(x_max),
        "y_min": np.asarray(y_min),
        "y_max": np.asarray(y_max),
    })


# revision 11
# speedup vs baseline: 1.4374x; 1.4374x over previous
"""BoxConv2d Trainium2 kernel.

Math: the reference computes, per output channel k = (c, f),
    out[b,k] = interp-row(I) diff  then  interp-col diff
where I is the zero-padded integral image of input[b,c].  The whole
pipeline (integral image + fractional box-edge interpolation) is linear
in the input and separable, so it collapses to

    out[b,k] = A_k @ x[b,c] @ B_k^T

with banded "pixel overlap" matrices
    A_k[xo, a] = clamp(xo + x_max_k + 1 - a, 0, 1) - clamp(xo + x_min_k - a, 0, 1)
    B_k[yo, j] = clamp(yo + y_max_k + 1 - j, 0, 1) - clamp(yo + y_min_k - j, 0, 1)

(A_k[xo, a] is exactly the length of the overlap between the box row
extent [xo + x_min, xo + x_max + 1] and the pixel row [a, a+1]; same for
columns.)  A/B are built on the host from the tiny (C,F) box params and
shipped per-core; the device does pure 128x128-contraction matmuls on
the PE array.

Device dataflow per core (16 output channels = 4 in_planes):
  pass 1 (per b,c):  V[j, (f,xo)]  = x_bc^T A_k^T   (lhsT=x_bc, rhs=A^T, N=512)
  pass 2 (per c,f,b-half): O[yo, (b,xo)] = B_k V    (lhsT=B_k^T, rhs=V, N=512)
Both passes stream 512 columns per matmul so float32r runs at full PE
rate.  Pass 2 produces the output TRANSPOSED (yo on partitions); it is
stored transposed in DRAM and the host untransposes when assembling.

Sharding: output channels K = C*F = 128 split across 8 cores.  Each
core reads only its own 4 input planes.
"""

import os
import sys

if "/opt/trn_rl_repo" not in sys.path:
    sys.path.insert(0, "/opt/trn_rl_repo")

import numpy as np

import concourse.bass as bass  # noqa: F401
import concourse.mybir as mybir
import concourse.tile as tile
from concourse import bacc
from concourse.bass_utils import run_bass_kernel_spmd

B, C, F, H, W = 8, 32, 4, 128, 128
NCORES = 8
CPC = C // NCORES  # in_planes per core
KPC = CPC * F      # output channels per core
BH = B // 2        # batch half

_DT = mybir.dt.float32
# matmul-operand dtype: float32r streams fp32 through the PE at full
# rate for N>=256 (vs 4 cycles/row for plain float32), at reduced
# multiply precision (~tf32).
_MM_DT = {
    "f32": mybir.dt.float32,
    "f32r": mybir.dt.float32r,
}[os.environ.get("BOXCONV_MM_DT", "f32r")]

_NC_CACHE = {}
LAST_RESULT = None


def _build_nc():
    nc = bacc.Bacc(
        "TRN2", target_bir_lowering=False, debug=False, num_devices=NCORES
    )
    x_p = nc.declare_dram_parameter("x", [B, H, CPC * W], _MM_DT, isOutput=False)
    at_p = nc.declare_dram_parameter("at", [H, KPC * H], _MM_DT, isOutput=False)
    bt_p = nc.declare_dram_parameter("bt", [W, KPC * W], _MM_DT, isOutput=False)
    # transposed output: outT[b, yo, kl*H + xo] = out[b, k, xo, yo]
    out_p = nc.declare_dram_parameter("outT", [B, W, KPC * H], _DT, isOutput=True)

    with tile.TileContext(nc) as tc:
        with (
            tc.tile_pool(name="const", bufs=1) as cpool,
            tc.tile_pool(name="xin", bufs=B) as xpool,
            tc.tile_pool(name="vall", bufs=2) as vpool,
            tc.tile_pool(name="osb", bufs=4) as opool,
            tc.tile_pool(name="pv", bufs=3, space="PSUM") as pvpool,
            tc.tile_pool(name="po", bufs=3, space="PSUM") as popool,
        ):
            at_sb = cpool.tile([128, KPC * H], _MM_DT, tag="at")
            nc.sync.dma_start(at_sb[:], at_p[:])
            bt_sb = cpool.tile([128, KPC * W], _MM_DT, tag="bt")
            nc.sync.dma_start(bt_sb[:], bt_p[:])

            x_sb = [None] * B
            for b in range(B):
                x_sb[b] = xpool.tile(
                    [128, CPC * W], _MM_DT, name=f"xsb{b}", tag="x"
                )
                nc.sync.dma_start(x_sb[b][:], x_p[b])

            for c in range(CPC):
                # V_all[j, (f, b, xo)]
                v_all = vpool.tile([128, F * B * H], _MM_DT)
                v_r = v_all[:].rearrange("p (f b xo) -> p f b xo", f=F, b=B)
                for b in range(B):
                    # pass 1: V[j, (f,xo)] = sum_a x[a, j] * A_k[xo, a]
                    v_ps = pvpool.tile([128, F * H], mybir.dt.float32)
                    nc.tensor.matmul(
                        v_ps[:],
                        lhsT=x_sb[b][:, c * W:(c + 1) * W],
                        rhs=at_sb[:, c * F * H:(c + 1) * F * H],
                        start=True,
                        stop=True,
                    )
                    # scatter the 4 f-blocks into V_all's (f, b, .) slots
                    nc.scalar.copy(v_r[:, :, b, :], v_ps[:])
                for f in range(F):
                    kl = c * F + f
                    for h in range(2):  # batch halves
                        # pass 2: O[yo, (b,xo)] = sum_j B_k[yo,j] * V[j, (b,xo)]
                        o_ps = popool.tile([128, BH * H], mybir.dt.float32)
                        nc.tensor.matmul(
                            o_ps[:],
                            lhsT=bt_sb[:, kl * W:(kl + 1) * W],
                            rhs=v_all[:, (f * B + h * BH) * H:(f * B + (h + 1) * BH) * H],
                            start=True,
                            stop=True,
                        )
                        o_sb = opool.tile([128, BH * H], _DT)
                        nc.vector.tensor_copy(o_sb[:], o_ps[:])
                        # dest: outT[b, :, kl*H : (kl+1)*H] for the 4 b's
                        dest = out_p[h * BH:(h + 1) * BH, :, kl * H:(kl + 1) * H]
                        dest = dest.rearrange("b yo xo -> yo b xo")
                        nc.sync.dma_start(dest, o_sb[:])
    nc.finalize()
    return nc


def _get_nc():
    if "nc" not in _NC_CACHE:
        _NC_CACHE["nc"] = _build_nc()
    return _NC_CACHE["nc"]


def _overlap_mats(lo, hi):
    """(K, out, in) pixel-overlap matrices for 128-wide axis."""
    t = np.arange(128, dtype=np.float64)
    d = t[:, None] - t[None, :]  # out - in
    lo = lo.astype(np.float64)[:, None, None]
    hi = hi.astype(np.float64)[:, None, None]
    m = np.clip(d[None] + hi + 1.0, 0.0, 1.0) - np.clip(d[None] + lo, 0.0, 1.0)
    return m.astype(np.float32)


def _make_in_maps(input, x_min, x_max, y_min, y_max):
    A = _overlap_mats(x_min.reshape(-1), x_max.reshape(-1))   # (K, xo, a)
    Bm = _overlap_mats(y_min.reshape(-1), y_max.reshape(-1))  # (K, yo, j)
    in_maps = []
    for m in range(NCORES):
        ks = slice(KPC * m, KPC * (m + 1))
        at = A[ks].transpose(2, 0, 1).reshape(H, KPC * H)     # [a, (kl, xo)]
        bt = Bm[ks].transpose(2, 0, 1).reshape(W, KPC * W)    # [j, (kl, yo)]
        xm = input[:, CPC * m:CPC * (m + 1)].transpose(0, 2, 1, 3)
        xm = xm.reshape(B, H, CPC * W)                        # [b, a, (c, j)]
        in_maps.append({
            "x": np.ascontiguousarray(xm, dtype=np.float32),
            "at": np.ascontiguousarray(at, dtype=np.float32),
            "bt": np.ascontiguousarray(bt, dtype=np.float32),
        })
    return in_maps


def _assemble(results):
    out = np.empty((B, C * F, H, W), np.float32)
    for m in range(NCORES):
        # outT[b, yo, kl*H+xo] -> out[b, k, xo, yo]
        o = results[m]["outT"].reshape(B, W, KPC, H).transpose(0, 2, 3, 1)
        out[:, KPC * m:KPC * (m + 1)] = o
    return out


def _run(inputs, trace=False):
    global LAST_RESULT
    nc = _get_nc()
    in_maps = _make_in_maps(**inputs)
    LAST_RESULT = run_bass_kernel_spmd(
        nc, in_maps, list(range(NCORES)), trace=trace
    )
    return _assemble(LAST_RESULT.results)


def kernel(input, x_min, x_max, y_min, y_max):
    return _run({
        "input": np.asarray(input),
        "x_min": np.asarray(x_min),
        "x_max": np.asarray(x_max),
        "y_min": np.asarray(y_min),
        "y_max": np.asarray(y_max),
    })


# revision 12
# speedup vs baseline: 1.4835x; 1.0321x over previous
"""BoxConv2d Trainium2 kernel.

Math: the reference computes, per output channel k = (c, f),
    out[b,k] = interp-row(I) diff  then  interp-col diff
where I is the zero-padded integral image of input[b,c].  The whole
pipeline (integral image + fractional box-edge interpolation) is linear
in the input and separable, so it collapses to

    out[b,k] = A_k @ x[b,c] @ B_k^T

with banded "pixel overlap" matrices
    A_k[xo, a] = clamp(xo + x_max_k + 1 - a, 0, 1) - clamp(xo + x_min_k - a, 0, 1)
    B_k[yo, j] = clamp(yo + y_max_k + 1 - j, 0, 1) - clamp(yo + y_min_k - j, 0, 1)

(A_k[xo, a] is exactly the length of the overlap between the box row
extent [xo + x_min, xo + x_max + 1] and the pixel row [a, a+1]; same for
columns.)  A/B are built on the host from the tiny (C,F) box params and
shipped per-core; the device does pure 128x128-contraction matmuls on
the PE array.

Device dataflow per core (16 output channels = 4 in_planes):
  pass 1 (per b,c):  V[j, (f,xo)]  = x_bc^T A_k^T   (lhsT=x_bc, rhs=A^T, N=512)
  pass 2 (per c,f,b-half): O[yo, (b,xo)] = B_k V    (lhsT=B_k^T, rhs=V, N=512)
Both passes stream 512 columns per matmul so float32r runs at full PE
rate.  Pass 2 produces the output TRANSPOSED (yo on partitions); it is
stored transposed in DRAM ([yo, kl, b, xo], 2KB-contiguous DMA lines)
and the host untransposes when assembling.  The c-loop is software
pipelined: pass 1 of plane c is emitted together with pass 2 of plane
c-1 so the PE never stalls on the PSUM->SBUF copies between passes.

Sharding: output channels K = C*F = 128 split across 8 cores.  Each
core reads only its own 4 input planes.
"""

import os
import sys

if "/opt/trn_rl_repo" not in sys.path:
    sys.path.insert(0, "/opt/trn_rl_repo")

import numpy as np

import concourse.bass as bass  # noqa: F401
import concourse.mybir as mybir
import concourse.tile as tile
from concourse import bacc
from concourse.bass_utils import run_bass_kernel_spmd

B, C, F, H, W = 8, 32, 4, 128, 128
NCORES = 8
CPC = C // NCORES  # in_planes per core
KPC = CPC * F      # output channels per core
BH = B // 2        # batch half

_DT = mybir.dt.float32
# matmul-operand dtype: float32r streams fp32 through the PE at full
# rate for N>=256 (vs 4 cycles/row for plain float32), at reduced
# multiply precision (~tf32).
_MM_DT = {
    "f32": mybir.dt.float32,
    "f32r": mybir.dt.float32r,
}[os.environ.get("BOXCONV_MM_DT", "f32r")]

_NC_CACHE = {}
LAST_RESULT = None


def _build_nc():
    nc = bacc.Bacc(
        "TRN2", target_bir_lowering=False, debug=False, num_devices=NCORES
    )
    x_p = nc.declare_dram_parameter("x", [B, H, CPC * W], _MM_DT, isOutput=False)
    at_p = nc.declare_dram_parameter("at", [CPC, H, F * H], _MM_DT, isOutput=False)
    bt_p = nc.declare_dram_parameter("bt", [CPC, W, F * W], _MM_DT, isOutput=False)
    # transposed output: outT[yo, kl, b, xo] = out[b, kl, xo, yo]
    out_p = nc.declare_dram_parameter("outT", [W, KPC, B, H], _DT, isOutput=True)

    with tile.TileContext(nc) as tc:
        with (
            tc.tile_pool(name="const", bufs=1) as cpool,
            tc.tile_pool(name="xin", bufs=B) as xpool,
            tc.tile_pool(name="vall", bufs=2) as vpool,
            tc.tile_pool(name="osb", bufs=4) as opool,
            tc.tile_pool(name="pv", bufs=3, space="PSUM") as pvpool,
            tc.tile_pool(name="po", bufs=3, space="PSUM") as popool,
        ):
            at_sb = [None] * CPC
            bt_sb = [None] * CPC
            for c in range(CPC):
                at_sb[c] = cpool.tile(
                    [128, F * H], _MM_DT, name=f"at{c}", tag=f"at{c}"
                )
                nc.sync.dma_start(at_sb[c][:], at_p[c])
                bt_sb[c] = cpool.tile(
                    [128, F * W], _MM_DT, name=f"bt{c}", tag=f"bt{c}"
                )
                nc.sync.dma_start(bt_sb[c][:], bt_p[c])

            x_sb = [None] * B
            for b in range(B):
                x_sb[b] = xpool.tile(
                    [128, CPC * W], _MM_DT, name=f"xsb{b}", tag="x"
                )
                nc.sync.dma_start(x_sb[b][:], x_p[b])

            v_alls = [None] * CPC

            def emit_pass1(c):
                # V_all[j, (f, b, xo)]
                v_all = vpool.tile([128, F * B * H], _MM_DT, name=f"vall{c}",
                                   tag="vall")
                v_alls[c] = v_all
                v_r = v_all[:].rearrange("p (f b xo) -> p f b xo", f=F, b=B)
                for b in range(B):
                    # V[j, (f,xo)] = sum_a x[a, j] * A_k[xo, a]
                    v_ps = pvpool.tile([128, F * H], mybir.dt.float32,
                                       name=f"vps{c}{b}", tag="vps")
                    nc.tensor.matmul(
                        v_ps[:],
                        lhsT=x_sb[b][:, c * W:(c + 1) * W],
                        rhs=at_sb[c][:],
                        start=True,
                        stop=True,
                    )
                    # scatter the 4 f-blocks into V_all's (f, b, .) slots
                    nc.scalar.copy(v_r[:, :, b, :], v_ps[:])

            def emit_pass2(c):
                v_all = v_alls[c]
                for f in range(F):
                    kl = c * F + f
                    for h in range(2):  # batch halves
                        # O[yo, (b,xo)] = sum_j B_k[yo,j] * V[j, (b,xo)]
                        o_ps = popool.tile([128, BH * H], mybir.dt.float32,
                                           name=f"ops{c}{f}{h}", tag="ops")
                        nc.tensor.matmul(
                            o_ps[:],
                            lhsT=bt_sb[c][:, f * W:(f + 1) * W],
                            rhs=v_all[:, (f * B + h * BH) * H:
                                      (f * B + (h + 1) * BH) * H],
                            start=True,
                            stop=True,
                        )
                        o_sb = opool.tile([128, BH * H], _DT,
                                          name=f"osb{c}{f}{h}", tag="osb")
                        nc.vector.tensor_copy(o_sb[:], o_ps[:])
                        # outT[:, kl, 4h:4h+4, :]; (b, xo) free dims are
                        # contiguous 2KB per partition line
                        nc.sync.dma_start(
                            out_p[:, kl, h * BH:(h + 1) * BH, :], o_sb[:]
                        )

            # software pipeline: pass1(c) || pass2(c-1)
            emit_pass1(0)
            for c in range(1, CPC):
                emit_pass1(c)
                emit_pass2(c - 1)
            emit_pass2(CPC - 1)
    nc.finalize()
    return nc


def _get_nc():
    if "nc" not in _NC_CACHE:
        _NC_CACHE["nc"] = _build_nc()
    return _NC_CACHE["nc"]


def _overlap_mats(lo, hi):
    """(K, out, in) pixel-overlap matrices for 128-wide axis."""
    t = np.arange(128, dtype=np.float64)
    d = t[:, None] - t[None, :]  # out - in
    lo = lo.astype(np.float64)[:, None, None]
    hi = hi.astype(np.float64)[:, None, None]
    m = np.clip(d[None] + hi + 1.0, 0.0, 1.0) - np.clip(d[None] + lo, 0.0, 1.0)
    return m.astype(np.float32)


def _make_in_maps(input, x_min, x_max, y_min, y_max):
    A = _overlap_mats(x_min.reshape(-1), x_max.reshape(-1))   # (K, xo, a)
    Bm = _overlap_mats(y_min.reshape(-1), y_max.reshape(-1))  # (K, yo, j)
    in_maps = []
    for m in range(NCORES):
        ks = slice(KPC * m, KPC * (m + 1))
        # at[c, a, (f, xo)] = A[k=c*F+f, xo, a]
        at = A[ks].reshape(CPC, F, H, H).transpose(0, 3, 1, 2)
        at = at.reshape(CPC, H, F * H)
        bt = Bm[ks].reshape(CPC, F, W, W).transpose(0, 3, 1, 2)
        bt = bt.reshape(CPC, W, F * W)
        xm = input[:, CPC * m:CPC * (m + 1)].transpose(0, 2, 1, 3)
        xm = xm.reshape(B, H, CPC * W)                        # [b, a, (c, j)]
        in_maps.append({
            "x": np.ascontiguousarray(xm, dtype=np.float32),
            "at": np.ascontiguousarray(at, dtype=np.float32),
            "bt": np.ascontiguousarray(bt, dtype=np.float32),
        })
    return in_maps


def _assemble(results):
    out = np.empty((B, C * F, H, W), np.float32)
    for m in range(NCORES):
        # outT[yo, kl, b, xo] -> out[b, kl, xo, yo]
        o = results[m]["outT"].transpose(2, 1, 3, 0)
        out[:, KPC * m:KPC * (m + 1)] = o
    return out


def _run(inputs, trace=False):
    global LAST_RESULT
    nc = _get_nc()
    in_maps = _make_in_maps(**inputs)
    LAST_RESULT = run_bass_kernel_spmd(
        nc, in_maps, list(range(NCORES)), trace=trace
    )
    return _assemble(LAST_RESULT.results)


def kernel(input, x_min, x_max, y_min, y_max):
    return _run({
        "input": np.asarray(input),
        "x_min": np.asarray(x_min),
        "x_max": np.asarray(x_max),
        "y_min": np.asarray(y_min),
        "y_max": np.asarray(y_max),
    })


# revision 13
# speedup vs baseline: 1.5723x; 1.0599x over previous
"""BoxConv2d Trainium2 kernel.

Math: the reference computes, per output channel k = (c, f),
    out[b,k] = interp-row(I) diff  then  interp-col diff
where I is the zero-padded integral image of input[b,c].  The whole
pipeline (integral image + fractional box-edge interpolation) is linear
in the input and separable, so it collapses to

    out[b,k] = A_k @ x[b,c] @ B_k^T

with banded "pixel overlap" matrices
    A_k[xo, a] = clamp(xo + x_max_k + 1 - a, 0, 1) - clamp(xo + x_min_k - a, 0, 1)
    B_k[yo, j] = clamp(yo + y_max_k + 1 - j, 0, 1) - clamp(yo + y_min_k - j, 0, 1)

(A_k[xo, a] is exactly the length of the overlap between the box row
extent [xo + x_min, xo + x_max + 1] and the pixel row [a, a+1]; same for
columns.)  A/B are built on the host from the tiny (C,F) box params and
shipped per-core; the device does pure 128x128-contraction matmuls on
the PE array.

Device dataflow per core (16 output channels = 4 in_planes):
  pass 1 (per b,c):  V[j, (f,xo)]  = x_bc^T A_k^T   (lhsT=x_bc, rhs=A^T, N=512)
  pass 2 (per c,f,b-half): O[yo, (b,xo)] = B_k V    (lhsT=B_k^T, rhs=V, N=512)
Both passes stream 512 columns per matmul so float32r runs at full PE
rate.  Pass 2 produces the output TRANSPOSED (yo on partitions); it is
stored transposed in DRAM ([yo, kl, b, xo], 2KB-contiguous DMA lines)
and the host untransposes when assembling.  The c-loop is software
pipelined: pass 1 of plane c is emitted together with pass 2 of plane
c-1 so the PE never stalls on the PSUM->SBUF copies between passes.

Sharding: output channels K = C*F = 128 split across 8 cores.  Each
core reads only its own 4 input planes.
"""

import os
import sys

if "/opt/trn_rl_repo" not in sys.path:
    sys.path.insert(0, "/opt/trn_rl_repo")

import numpy as np

import concourse.bass as bass  # noqa: F401
import concourse.mybir as mybir
import concourse.tile as tile
from concourse import bacc
from concourse.bass_utils import run_bass_kernel_spmd

B, C, F, H, W = 8, 32, 4, 128, 128
NCORES = 8
CPC = C // NCORES  # in_planes per core
KPC = CPC * F      # output channels per core
BH = B // 2        # batch half

_DT = mybir.dt.float32
# matmul-operand dtype: float32r streams fp32 through the PE at full
# rate for N>=256 (vs 4 cycles/row for plain float32), at reduced
# multiply precision (~tf32).
_MM_DT = {
    "f32": mybir.dt.float32,
    "f32r": mybir.dt.float32r,
}[os.environ.get("BOXCONV_MM_DT", "f32r")]

_NC_CACHE = {}
LAST_RESULT = None


def _build_nc():
    nc = bacc.Bacc(
        "TRN2", target_bir_lowering=False, debug=False, num_devices=NCORES
    )
    x_p = nc.declare_dram_parameter("x", [B, H, CPC * W], _MM_DT, isOutput=False)
    at_p = nc.declare_dram_parameter("at", [CPC, H, F * H], _MM_DT, isOutput=False)
    bt_p = nc.declare_dram_parameter("bt", [CPC, W, F * W], _MM_DT, isOutput=False)
    # transposed output: outT[yo, kl, b, xo] = out[b, kl, xo, yo]
    out_p = nc.declare_dram_parameter("outT", [W, KPC, B, H], _DT, isOutput=True)

    with tile.TileContext(nc) as tc:
        with (
            tc.tile_pool(name="const", bufs=1) as cpool,
            tc.tile_pool(name="xin", bufs=B) as xpool,
            tc.tile_pool(name="vall", bufs=2) as vpool,
            tc.tile_pool(name="osb", bufs=4) as opool,
            tc.tile_pool(name="pv", bufs=3, space="PSUM") as pvpool,
            tc.tile_pool(name="po", bufs=3, space="PSUM") as popool,
        ):
            at_sb = [None] * CPC
            bt_sb = [None] * CPC
            x_sb = [None] * B

            def load_at(c):
                at_sb[c] = cpool.tile(
                    [128, F * H], _MM_DT, name=f"at{c}", tag=f"at{c}"
                )
                nc.sync.dma_start(at_sb[c][:], at_p[c])

            def load_bt(c):
                bt_sb[c] = cpool.tile(
                    [128, F * W], _MM_DT, name=f"bt{c}", tag=f"bt{c}"
                )
                nc.sync.dma_start(bt_sb[c][:], bt_p[c])

            def load_x(b):
                x_sb[b] = xpool.tile(
                    [128, CPC * W], _MM_DT, name=f"xsb{b}", tag="x"
                )
                nc.sync.dma_start(x_sb[b][:], x_p[b])

            # order loads so pass1(c=0) can start as early as possible
            load_x(0)
            load_at(0)
            for b in range(1, 4):
                load_x(b)
            load_at(1)
            load_bt(0)
            for b in range(4, B):
                load_x(b)
            for c in range(2, CPC):
                load_at(c)
            for c in range(1, CPC):
                load_bt(c)

            v_alls = [None] * CPC

            def emit_pass1(c):
                # V_all[j, (f, b, xo)]
                v_all = vpool.tile([128, F * B * H], _MM_DT, name=f"vall{c}",
                                   tag="vall")
                v_alls[c] = v_all
                v_r = v_all[:].rearrange("p (f b xo) -> p f b xo", f=F, b=B)
                for b in range(B):
                    # V[j, (f,xo)] = sum_a x[a, j] * A_k[xo, a]
                    v_ps = pvpool.tile([128, F * H], mybir.dt.float32,
                                       name=f"vps{c}{b}", tag="vps")
                    nc.tensor.matmul(
                        v_ps[:],
                        lhsT=x_sb[b][:, c * W:(c + 1) * W],
                        rhs=at_sb[c][:],
                        start=True,
                        stop=True,
                    )
                    # scatter the 4 f-blocks into V_all's (f, b, .) slots
                    nc.scalar.copy(v_r[:, :, b, :], v_ps[:])

            def emit_pass2(c):
                v_all = v_alls[c]
                for f in range(F):
                    kl = c * F + f
                    for h in range(2):  # batch halves
                        # O[yo, (b,xo)] = sum_j B_k[yo,j] * V[j, (b,xo)]
                        o_ps = popool.tile([128, BH * H], mybir.dt.float32,
                                           name=f"ops{c}{f}{h}", tag="ops")
                        nc.tensor.matmul(
                            o_ps[:],
                            lhsT=bt_sb[c][:, f * W:(f + 1) * W],
                            rhs=v_all[:, (f * B + h * BH) * H:
                                      (f * B + (h + 1) * BH) * H],
                            start=True,
                            stop=True,
                        )
                        o_sb = opool.tile([128, BH * H], _DT,
                                          name=f"osb{c}{f}{h}", tag="osb")
                        nc.vector.tensor_copy(o_sb[:], o_ps[:])
                        # outT[:, kl, 4h:4h+4, :]; (b, xo) free dims are
                        # contiguous 2KB per partition line
                        nc.sync.dma_start(
                            out_p[:, kl, h * BH:(h + 1) * BH, :], o_sb[:]
                        )

            # software pipeline: pass1(c) || pass2(c-1)
            emit_pass1(0)
            for c in range(1, CPC):
                emit_pass1(c)
                emit_pass2(c - 1)
            emit_pass2(CPC - 1)
    nc.finalize()
    return nc


def _get_nc():
    if "nc" not in _NC_CACHE:
        _NC_CACHE["nc"] = _build_nc()
    return _NC_CACHE["nc"]


def _overlap_mats(lo, hi):
    """(K, out, in) pixel-overlap matrices for 128-wide axis."""
    t = np.arange(128, dtype=np.float64)
    d = t[:, None] - t[None, :]  # out - in
    lo = lo.astype(np.float64)[:, None, None]
    hi = hi.astype(np.float64)[:, None, None]
    m = np.clip(d[None] + hi + 1.0, 0.0, 1.0) - np.clip(d[None] + lo, 0.0, 1.0)
    return m.astype(np.float32)


def _make_in_maps(input, x_min, x_max, y_min, y_max):
    A = _overlap_mats(x_min.reshape(-1), x_max.reshape(-1))   # (K, xo, a)
    Bm = _overlap_mats(y_min.reshape(-1), y_max.reshape(-1))  # (K, yo, j)
    in_maps = []
    for m in range(NCORES):
        ks = slice(KPC * m, KPC * (m + 1))
        # at[c, a, (f, xo)] = A[k=c*F+f, xo, a]
        at = A[ks].reshape(CPC, F, H, H).transpose(0, 3, 1, 2)
        at = at.reshape(CPC, H, F * H)
        bt = Bm[ks].reshape(CPC, F, W, W).transpose(0, 3, 1, 2)
        bt = bt.reshape(CPC, W, F * W)
        xm = input[:, CPC * m:CPC * (m + 1)].transpose(0, 2, 1, 3)
        xm = xm.reshape(B, H, CPC * W)                        # [b, a, (c, j)]
        in_maps.append({
            "x": np.ascontiguousarray(xm, dtype=np.float32),
            "at": np.ascontiguousarray(at, dtype=np.float32),
            "bt": np.ascontiguousarray(bt, dtype=np.float32),
        })
    return in_maps


def _assemble(results):
    out = np.empty((B, C * F, H, W), np.float32)
    for m in range(NCORES):
        # outT[yo, kl, b, xo] -> out[b, kl, xo, yo]
        o = results[m]["outT"].transpose(2, 1, 3, 0)
        out[:, KPC * m:KPC * (m + 1)] = o
    return out


def _run(inputs, trace=False):
    global LAST_RESULT
    nc = _get_nc()
    in_maps = _make_in_maps(**inputs)
    LAST_RESULT = run_bass_kernel_spmd(
        nc, in_maps, list(range(NCORES)), trace=trace
    )
    return _assemble(LAST_RESULT.results)


def kernel(input, x_min, x_max, y_min, y_max):
    return _run({
        "input": np.asarray(input),
        "x_min": np.asarray(x_min),
        "x_max": np.asarray(x_max),
        "y_min": np.asarray(y_min),
        "y_max": np.asarray(y_max),
    })
